# revision 48
# baseline (speedup 1.0000x reference)
"""DepthAwareBokehDFN Trainium2 kernel (v2: row-pair matmuls).

Network (per image): x = concat(rgb, depth) (4ch) -> conv3x3(64)+relu ->
conv3x3(64)+relu -> conv3x3(81) -> softmax over 81 taps -> 9x9 dynamic
filtering of rgb.

Distribution: pure data parallel over 8 cores; shard = (batch, H-half),
192 output rows per core.  Halos recomputed from DRAM (no collectives).

v2 dataflow (row-PAIR streamed; all conv outputs produced two rows per
PSUM tile, M = 64ch x 2rows = 128):
  - conv1: host-side im2col with the pair structure baked in (x74: 36
    taps x 2 row-offsets + 2 bias/ones channels) -> ONE matmul per row
    pair (K=74, N=384).  Out-of-image rows are zeroed host-side
    (including the ones channel), so padding rows come out exactly 0.
  - conv2: 6 matmuls per row pair (3 kw x 2 input row-pairs, K=128);
    output partitions (co, j) land both window halves of one slot in a
    single ACT evac -> the per-row window-fill DMAs of v1 are gone.
    Image-boundary output rows (-1 / R) are zeroed by a per-core mask
    multiply (DVE), restoring SAME-pad semantics.
  - conv3: per row, 3 full-K (paired rows) + 3 half-K matmuls into a
    2-row PSUM tile; one bulk 2-row exp evac (ACT, bias=b3).
  - softmax denominator folded into the bokeh tree as a 4th channel.
  - bokeh in "y-layout" (partition = half-row), rgb prescattered on the
    host with per-tap shifts so every tap is a free-dim offset; products
    on DVE in bf16 (2x mode), pairwise adder tree (L1/L2 bf16, rest
    fp32); E reshaped into y-layout by per-row scatter DMAs (SWDGE).
"""

import os
import sys
import numpy as np

if "/opt/trn_rl_repo" not in sys.path:
    sys.path.insert(0, "/opt/trn_rl_repo")

import ml_dtypes  # noqa: E402
import concourse.bass as bass  # noqa: E402
import concourse.bacc as bacc  # noqa: E402
import concourse.mybir as mybir  # noqa: E402
import concourse.tile as tile  # noqa: E402

F32 = mybir.dt.float32
F32R = mybir.dt.float32r
BF16 = mybir.dt.bfloat16

B, H, W = 4, 384, 384
NC_ = 8         # cores
RS = 392        # row slot stride (elements) in window / x74 buffers
WPAD = 400      # rgb halo padded width
HW2 = 192       # half-row width
SR = 64         # rows per bokeh strip (=> 128 half-rows = 128 partitions)
NS = 6          # pair slots in h1w/h2w rings
XS = 8          # pair slots in x74 window
RGBF = 27 * 200  # rgb halo block elems per partition (3ch * 9dy * 200)

# bf16 weight table columns: l1 (74,128) | l2lo[3] | l2hi[3] (each 128x128)
C_L1 = 0
C_L2LO = 128
C_L2HI = C_L2LO + 384
WB_COLS = C_L2HI + 384          # 896
# f32r table: l3 full-K [even|odd][kw] 6x(128,81) then half-K:
#   even rows: kh2 at parts 0:64 (3x81); odd rows: kh0 at parts 64:128
C_L3F = 0
C_L3HE = 486
C_L3HO = 486 + 243
WR_COLS = 486 + 486             # 972


def build_core_program(R=192):
    """Builds the per-core Bass program.  R = output rows per core."""
    assert R % 2 == 0
    sr = min(SR, R)
    assert R % sr == 0
    nstrip = R // sr
    NP = (R + 4) // 2            # conv1 pairs: y = 2p-2, p in 0..NP-1

    nc = bacc.Bacc("TRN2", debug=False, enable_asserts=False,
                   num_devices=NC_, enable_partition_id=False,
                   num_swdge_queues=4)

    x74d = nc.dram_tensor("x74d", [74, NP, RS], BF16,
                          kind="ExternalInput").ap()
    rgbsA = nc.dram_tensor("rgbsA", [nstrip * 128, RGBF], BF16,
                           kind="ExternalInput").ap()
    rgbsB = nc.dram_tensor("rgbsB", [nstrip * 128, RGBF], BF16,
                           kind="ExternalInput").ap()
    wtsb = nc.dram_tensor("wtsb", [128, WB_COLS], BF16,
                          kind="ExternalInput").ap()
    wtsr = nc.dram_tensor("wtsr", [128, WR_COLS], BF16,
                          kind="ExternalInput").ap()
    wtb = nc.dram_tensor("wtb", [128, 4], F32, kind="ExternalInput").ap()
    maskd = nc.dram_tensor("maskd", [128, 384], F32,
                           kind="ExternalInput").ap()
    out = nc.dram_tensor("out", [3, R, W], F32, kind="ExternalOutput").ap()

    def sig1(y):   # h1w pair-slot of conv1 pair starting at even row y
        return ((y + 2) // 2) % NS

    def sig2(w0):  # h2w pair-slot of conv2 pair starting at odd row w0
        return ((w0 + 1) // 2) % NS

    AF = mybir.ActivationFunctionType

    with tile.TileContext(nc) as tc:
        with (
            tc.tile_pool(name="singles", bufs=1) as singles,
            tc.tile_pool(name="estg_pool", bufs=12) as estg_pool,
            tc.tile_pool(name="outstg_pool", bufs=1) as outstg_pool,
            tc.tile_pool(name="psum", bufs=1, space="PSUM") as psum,
        ):
            # ---- persistent SBUF state ----
            wtsb_sb = singles.tile([128, WB_COLS], BF16)
            nc.sync.dma_start(out=wtsb_sb, in_=wtsb)
            wtsr_sb = singles.tile([128, WR_COLS], BF16)
            nc.sync.dma_start(out=wtsr_sb, in_=wtsr)
            wtb_sb = singles.tile([128, 4], F32)
            nc.sync.dma_start(out=wtb_sb, in_=wtb)
            mask_sb = singles.tile([128, 384], F32)
            nc.sync.dma_start(out=mask_sb, in_=maskd)
            h1w = singles.tile([128, NS * RS], BF16)
            h2w = singles.tile([128, NS * RS], BF16)
            x74w = singles.tile([74, XS * RS], BF16)
            ebuf = [singles.tile([128, 81 * HW2], BF16, name=f"ebuf{i}")
                    for i in range(2)]
            rgbAB = [(singles.tile([128, RGBF], BF16, name=f"rgbA{i}"),
                      singles.tile([128, RGBF], BF16, name=f"rgbB{i}"))
                     for i in range(2)]
            tmpP = singles.tile([128, 81 * HW2], BF16)
            scrA = singles.tile([128, 40 * HW2], BF16)
            scrF = singles.tile([128, 19, HW2], F32)
            uacc = singles.tile([128, 4, HW2], F32)

            nc.vector.memset(h1w, 0.0)
            nc.vector.memset(h2w, 0.0)

            # weight slices
            l1 = wtsb_sb[0:74, C_L1:C_L1 + 128]
            l2lo = [wtsb_sb[0:128, C_L2LO + 128 * k:C_L2LO + 128 * (k + 1)]
                    for k in range(3)]
            l2hi = [wtsb_sb[0:128, C_L2HI + 128 * k:C_L2HI + 128 * (k + 1)]
                    for k in range(3)]
            # conv3 full-K: po=0 (even v), po=1 (odd v)
            l3f = [[wtsr_sb[0:128, C_L3F + 81 * (3 * po + k):
                            C_L3F + 81 * (3 * po + k + 1)]
                    for k in range(3)] for po in range(2)]
            l3he = [wtsr_sb[0:64, C_L3HE + 81 * k:C_L3HE + 81 * (k + 1)]
                    for k in range(3)]
            l3ho = [wtsr_sb[64:128, C_L3HO + 81 * k:C_L3HO + 81 * (k + 1)]
                    for k in range(3)]

            b2 = wtb_sb[0:128, 0:1]
            b3 = wtb_sb[0:81, 1:2]
            b2top = wtb_sb[0:128, 2:3]   # b2 with row -1 (parts 0:64) masked
            b2bot = wtb_sb[0:128, 3:4]   # b2 with row R (parts 64:128) masked

            # ---------------- emission helpers ----------------
            def emit_x74_batch(p0):
                # load x74 pair-slots p0..p0+3 in one DMA
                n = min(4, NP - p0)
                F = XS * RS
                dst = bass.AP(tensor=x74w.tensor, offset=(p0 % XS) * RS,
                              ap=[[F, 74], [RS, n], [1, RS]])
                src = bass.AP(tensor=x74d.tensor, offset=p0 * RS,
                              ap=[[NP * RS, 74], [RS, n], [1, RS]])
                # scalar queue: x74 loads never carry long waits, and
                # the sync queue may be parked on an out-DMA's RAW wait
                nc.scalar.dma_start(out=dst, in_=src)

            def emit_reshape(v, estg, idx):
                # scatter E row v (81, 384) bf16 into ebuf strip layout;
                # one DMA per half-row (dst = one partition)
                s, p0 = v // sr, 2 * (v % sr)
                eb = ebuf[s % 2]
                for h, eng in ((0, nc.scalar), (1, nc.gpsimd)):
                    eng.dma_start(
                        out=eb[p0 + h:p0 + h + 1, :],
                        in_=estg[0:81, idx, h * HW2:(h + 1) * HW2])

            def emit_rgb_dma(s):
                # double-buffered: HWDGE DMAs must never carry a long
                # wait (a parked DMA poisons its shared DMAHW sem lane
                # for every other waiter on that lane)
                ta, tb = rgbAB[s % 2]
                for t, srct in ((ta, rgbsA), (tb, rgbsB)):
                    nc.sync.dma_start(
                        out=t[0:2 * sr, :],
                        in_=srct[s * 128:s * 128 + 2 * sr, :])

            def emit_bokeh(s):
                np_ = 2 * sr  # partitions used
                rgbA, rgbB = rgbAB[s % 2]
                eb = ebuf[s % 2]
                EB = 81 * HW2
                SA = 40 * HW2
                ostg = outstg_pool.tile([128, 3, HW2], F32, name=f"ostg{s}",
                                        tag="ostg")

                with nc.allow_low_precision("bokeh bf16 tree by design"):
                    for ch in range(4):
                        if ch < 3:
                            # products tmpP[(dy,dx),x] = E * rgb_shift
                            for par, t in ((0, rgbA), (1, rgbB)):
                                tn = 5 - par  # 5 even-dx taps, 4 odd
                                dst = bass.AP(
                                    tensor=tmpP.tensor, offset=par * HW2,
                                    ap=[[EB, np_], [9 * HW2, 9],
                                        [2 * HW2, tn], [1, HW2]])
                                ein = bass.AP(
                                    tensor=eb.tensor, offset=par * HW2,
                                    ap=[[EB, np_], [9 * HW2, 9],
                                        [2 * HW2, tn], [1, HW2]])
                                rin = bass.AP(
                                    tensor=t.tensor, offset=ch * 1800,
                                    ap=[[RGBF, np_], [200, 9], [2, tn],
                                        [1, HW2]])
                                nc.vector.tensor_mul(dst, ein, rin)
                            src_t, SRCF = tmpP, EB
                        else:
                            src_t, SRCF = eb, EB

                        # pairwise tree over the 81 tap planes:
                        # L1: (0..79) -> scrA 0..39          (bf16)
                        nc.vector.tensor_add(
                            bass.AP(tensor=scrA.tensor, offset=0,
                                    ap=[[SA, np_], [HW2, 40], [1, HW2]]),
                            bass.AP(tensor=src_t.tensor, offset=0,
                                    ap=[[SRCF, np_], [2 * HW2, 40],
                                        [1, HW2]]),
                            bass.AP(tensor=src_t.tensor, offset=HW2,
                                    ap=[[SRCF, np_], [2 * HW2, 40],
                                        [1, HW2]]))
                        # L2: scrA 0..39 -> tmpP 0..19       (bf16)
                        nc.vector.tensor_add(
                            bass.AP(tensor=tmpP.tensor, offset=0,
                                    ap=[[EB, np_], [HW2, 20], [1, HW2]]),
                            bass.AP(tensor=scrA.tensor, offset=0,
                                    ap=[[SA, np_], [2 * HW2, 20], [1, HW2]]),
                            bass.AP(tensor=scrA.tensor, offset=HW2,
                                    ap=[[SA, np_], [2 * HW2, 20], [1, HW2]]))
                        # L3: tmpP 0..19 -> scrF 0..9        (fp32 out)
                        SF = 19 * HW2
                        nc.vector.tensor_add(
                            bass.AP(tensor=scrF.tensor, offset=0,
                                    ap=[[SF, np_], [HW2, 10], [1, HW2]]),
                            bass.AP(tensor=tmpP.tensor, offset=0,
                                    ap=[[EB, np_], [2 * HW2, 10], [1, HW2]]),
                            bass.AP(tensor=tmpP.tensor, offset=HW2,
                                    ap=[[EB, np_], [2 * HW2, 10], [1, HW2]]))
                        # L4: scrF 0..9 -> scrF 10..14
                        nc.vector.tensor_add(
                            bass.AP(tensor=scrF.tensor, offset=10 * HW2,
                                    ap=[[SF, np_], [HW2, 5], [1, HW2]]),
                            bass.AP(tensor=scrF.tensor, offset=0,
                                    ap=[[SF, np_], [2 * HW2, 5], [1, HW2]]),
                            bass.AP(tensor=scrF.tensor, offset=HW2,
                                    ap=[[SF, np_], [2 * HW2, 5], [1, HW2]]))
                        # L5: scrF 10..13 -> scrF 15..16  (leftover 14)
                        nc.vector.tensor_add(
                            bass.AP(tensor=scrF.tensor, offset=15 * HW2,
                                    ap=[[SF, np_], [HW2, 2], [1, HW2]]),
                            bass.AP(tensor=scrF.tensor, offset=10 * HW2,
                                    ap=[[SF, np_], [2 * HW2, 2], [1, HW2]]),
                            bass.AP(tensor=scrF.tensor, offset=11 * HW2,
                                    ap=[[SF, np_], [2 * HW2, 2], [1, HW2]]))
                        # L6: 15+16 -> 17 ; L7: 17+14 -> 18
                        nc.vector.tensor_add(scrF[0:np_, 17, :],
                                             scrF[0:np_, 15, :],
                                             scrF[0:np_, 16, :])
                        nc.vector.tensor_add(scrF[0:np_, 18, :],
                                             scrF[0:np_, 17, :],
                                             scrF[0:np_, 14, :])
                        # L8: + plane 80 (bf16 leftover) -> uacc[ch]
                        last = bass.AP(tensor=src_t.tensor, offset=80 * HW2,
                                       ap=[[SRCF, np_], [1, HW2]])
                        nc.vector.tensor_add(uacc[0:np_, ch, :],
                                             scrF[0:np_, 18, :], last)

                    # out = U * (1/S)
                    nc.vector.reciprocal(uacc[0:np_, 3, :], uacc[0:np_, 3, :])
                    for ch in range(3):
                        nc.vector.tensor_mul(ostg[0:np_, ch, :],
                                             uacc[0:np_, ch, :],
                                             uacc[0:np_, 3, :])

                # DMA strip output to DRAM (one DMA per channel).
                # SWDGE (gpsimd): its bokeh-long RAW wait must not park
                # an HWDGE ring/sem lane shared with the scatters; the
                # brief gpsimd-ring blockage is absorbed by the deep
                # estg pool.
                for ch in range(3):
                    dst = bass.AP(tensor=out.tensor,
                                  offset=ch * R * W + s * sr * W,
                                  ap=[[W, sr], [HW2, 2], [1, HW2]])
                    srcap = bass.AP(tensor=ostg.tensor, offset=ch * HW2,
                                    ap=[[3 * HW2, np_], [1, HW2]])
                    nc.gpsimd.dma_start(out=dst, in_=srcap)

            # ---------------- main row-pair loop ----------------
            # deep skew: conv2 lags conv1 by 2 pair-iterations, conv3
            # lags conv2 by 2 more, so each stage's inputs were evac'd
            # >=2 iterations ago and PE never waits on the previous
            # pair's ACT evac
            for k in range((R + 16) // 2):
                y = 2 * k - 2           # conv1 pair (y, y+1)
                if 1 <= k <= 8:
                    # strip-0 rgb prefetch, split into 16-partition
                    # chunks across iterations: one monolithic 2.76MB
                    # SWDGE DMA stalls every early waiter whose shared
                    # sem lane counts its completion (~18us startup
                    # stall); small chunks advance the lanes steadily
                    j = k - 1
                    for t_, srct in ((rgbAB[0][0], rgbsA),
                                     (rgbAB[0][1], rgbsB)):
                        nc.gpsimd.dma_start(
                            out=t_[16 * j:16 * (j + 1), :],
                            in_=srct[16 * j:16 * (j + 1), :])
                if y <= R:
                    if k == 0:
                        emit_x74_batch(0)
                    if k % 4 == 0 and k + 4 < NP:
                        emit_x74_batch(k + 4)
                    ps1 = psum.tile([128, 512], F32, tag=f"c1{k % 2}",
                                    name=f"c1_{k}")
                    rhs = x74w[0:74, (k % XS) * RS + 1:(k % XS) * RS + 385]
                    nc.tensor.matmul(out=ps1[0:128, 0:384], lhsT=l1,
                                     rhs=rhs, start=True, stop=True)
                    nc.scalar.activation(
                        out=h1w[0:128, sig1(y) * RS + 1:sig1(y) * RS + 385],
                        in_=ps1[0:128, 0:384], func=AF.Relu)

                # conv2 pair (w0, w0+1), w0 odd
                w0 = y - 5
                if -1 <= w0 <= R - 1:
                    ps2 = psum.tile([128, 512], F32, tag=f"c2{k % 2}",
                                    name=f"c2_{k}")
                    slo, shi = sig1(w0 - 1), sig1(w0 + 1)
                    for kw in range(3):
                        nc.tensor.matmul(
                            out=ps2[0:128, 0:384], lhsT=l2lo[kw],
                            rhs=h1w[0:128, slo * RS + kw:slo * RS + kw + 384],
                            start=(kw == 0), stop=False)
                    for kw in range(3):
                        nc.tensor.matmul(
                            out=ps2[0:128, 0:384], lhsT=l2hi[kw],
                            rhs=h1w[0:128, shi * RS + kw:shi * RS + kw + 384],
                            start=False, stop=(kw == 2))
                    # image-boundary SAME-pad fix: mask rows -1 / R in
                    # PSUM before the evac (bias also masked, so padding
                    # rows come out exactly relu(0) = 0)
                    bias2 = b2
                    if w0 == -1:
                        nc.vector.tensor_mul(ps2[0:64, 0:384],
                                             ps2[0:64, 0:384],
                                             mask_sb[0:64, 0:384])
                        bias2 = b2top
                    if w0 == R - 1:
                        nc.vector.tensor_mul(ps2[64:128, 0:384],
                                             ps2[64:128, 0:384],
                                             mask_sb[64:128, 0:384])
                        bias2 = b2bot
                    s2 = sig2(w0)
                    nc.scalar.activation(
                        out=h2w[0:128, s2 * RS + 1:s2 * RS + 385],
                        in_=ps2[0:128, 0:384], func=AF.Relu, bias=bias2)

                # conv3 rows (y-12, y-11)
                v0 = y - 12
                if 0 <= v0 <= R - 2:
                    ps3 = psum.tile([81, 2, 512], F32, tag=f"c3{k % 2}",
                                    name=f"c3_{k}")
                    for idx, v in enumerate((v0, v0 + 1)):
                        if v % sr == 0 and v > 0:
                            emit_rgb_dma(v // sr)
                        po = v % 2
                        pstart = v - 1 + po   # full-K pair start (odd)
                        sf = sig2(pstart)
                        outap = ps3[0:81, idx, 0:384]
                        for kw in range(3):
                            nc.tensor.matmul(
                                out=outap, lhsT=l3f[po][kw],
                                rhs=h2w[0:128,
                                        sf * RS + kw:sf * RS + kw + 384],
                                start=(kw == 0), stop=False)
                        if po == 0:   # even v: kh2 = row v+1 at lo half
                            sh = sig2(v + 1)
                            for kw in range(3):
                                nc.tensor.matmul(
                                    out=outap, lhsT=l3he[kw],
                                    rhs=h2w[0:64, sh * RS + kw:
                                            sh * RS + kw + 384],
                                    start=False, stop=(kw == 2))
                        else:         # odd v: kh0 = row v-1 at hi half
                            sh = sig2(v - 2)
                            for kw in range(3):
                                nc.tensor.matmul(
                                    out=outap, lhsT=l3ho[kw],
                                    rhs=h2w[64:128, sh * RS + kw:
                                            sh * RS + kw + 384],
                                    start=False, stop=(kw == 2))
                    estg = estg_pool.tile([81, 2, 384], BF16,
                                          name=f"estg_{k}", tag="estg")
                    nc.scalar.activation(out=estg[0:81, :, :],
                                         in_=ps3[0:81, 0:2, 0:384],
                                         func=AF.Exp, bias=b3)
                    for idx, v in enumerate((v0, v0 + 1)):
                        emit_reshape(v, estg, idx)
                        if v % sr == sr - 1:
                            emit_bokeh(v // sr)

    nc.compile()
    return nc


# ------------------------- host side -------------------------

def prep_weights(w1, b1, w2, b2, w3, b3):
    wtsb = np.zeros((128, WB_COLS), np.float32)
    # l1: K = (t, j) 72 + 2 bias channels; M = (co, j')
    for kh in range(3):
        for kw in range(3):
            for c in range(4):
                t = kh * 12 + kw * 4 + c
                wtsb[t, C_L1:C_L1 + 64] = w1[:, c, kh, kw]
                wtsb[36 + t, C_L1 + 64:C_L1 + 128] = w1[:, c, kh, kw]
    wtsb[72, C_L1:C_L1 + 64] = b1
    wtsb[73, C_L1 + 64:C_L1 + 128] = b1
    # conv2: lo pair (w0-1, w0): kh = a - j; hi pair (w0+1, w0+2): a - j + 2
    for kw in range(3):
        lo = np.zeros((128, 128), np.float32)
        hi = np.zeros((128, 128), np.float32)
        for a in range(2):
            for j in range(2):
                khl = a - j
                if 0 <= khl <= 2:
                    lo[a * 64:(a + 1) * 64, j * 64:(j + 1) * 64] = \
                        w2[:, :, khl, kw].T
                khh = a - j + 2
                if 0 <= khh <= 2:
                    hi[a * 64:(a + 1) * 64, j * 64:(j + 1) * 64] = \
                        w2[:, :, khh, kw].T
        wtsb[:, C_L2LO + 128 * kw:C_L2LO + 128 * (kw + 1)] = lo
        wtsb[:, C_L2HI + 128 * kw:C_L2HI + 128 * (kw + 1)] = hi
    # conv3 f32r tables
    wtsr = np.zeros((128, WR_COLS), np.float32)
    for kw in range(3):
        # even v: pair (v-1, v): kh = a
        c0 = C_L3F + 81 * kw
        wtsr[0:64, c0:c0 + 81] = w3[:, :, 0, kw].T
        wtsr[64:128, c0:c0 + 81] = w3[:, :, 1, kw].T
        # odd v: pair (v, v+1): kh = a + 1
        c1 = C_L3F + 81 * (3 + kw)
        wtsr[0:64, c1:c1 + 81] = w3[:, :, 1, kw].T
        wtsr[64:128, c1:c1 + 81] = w3[:, :, 2, kw].T
        # half-K: even v: kh2 at parts 0:64; odd v: kh0 at parts 64:128
        wtsr[0:64, C_L3HE + 81 * kw:C_L3HE + 81 * (kw + 1)] = \
            w3[:, :, 2, kw].T
        wtsr[64:128, C_L3HO + 81 * kw:C_L3HO + 81 * (kw + 1)] = \
            w3[:, :, 0, kw].T
    wtb = np.zeros((128, 4), np.float32)
    wtb[0:64, 0] = b2
    wtb[64:128, 0] = b2
    wtb[0:81, 1] = b3
    wtb[:, 2] = wtb[:, 0]
    wtb[:, 3] = wtb[:, 0]
    return (wtsb.astype(ml_dtypes.bfloat16),
            wtsr.astype(ml_dtypes.bfloat16), wtb)


def prep_shard(x, rgb_b, r0, R):
    """x: (4,H,W) fp32 of one image; rgb_b: (3,H,W).

    Returns (x74d, rgbsA, rgbsB, maskd)."""
    NP = (R + 4) // 2
    # padded x rows r0-3 .. r0+R+3, width 392 (img col x at 1+x)
    xp = np.zeros((4, R + 6, RS), np.float32)
    lo, hi = r0 - 3, r0 + R + 3
    slo, shi = max(lo, 0), min(hi, H)
    xp[:, slo - lo:shi - lo, 1:385] = x[:, slo:shi, :]
    # x36 rows j = conv1 out rows -2 .. R+1 (index j+2)
    x36 = np.zeros((36, R + 4, RS), np.float32)
    for kh in range(3):
        for kw in range(3):
            blk = np.zeros((4, R + 4, RS), np.float32)
            if kw == 0:
                blk[:, :, 1:] = xp[:, kh:kh + R + 4, :-1]
            elif kw == 1:
                blk[:, :, :] = xp[:, kh:kh + R + 4, :]
            else:
                blk[:, :, :-1] = xp[:, kh:kh + R + 4, 1:]
            for c in range(4):
                x36[kh * 12 + kw * 4 + c] = blk[c]
    # in-image mask for conv1 out rows (global row in [0, H))
    def inimg(j):  # j = local conv1-out row
        g = r0 + j
        return 0 <= g < H
    x74 = np.zeros((74, NP, RS), np.float32)
    for p in range(NP):
        for j in range(2):
            row = 2 * p - 2 + j
            if inimg(row):
                x74[36 * j:36 * (j + 1), p, :] = x36[:, row + 2, :]
                x74[72 + j, p, 1:385] = 1.0
    x74d = x74.astype(ml_dtypes.bfloat16)

    maskd = np.zeros((128, 384), np.float32)
    maskd[0:64, :] = 1.0 if r0 - 1 >= 0 else 0.0      # conv2 row -1
    maskd[64:128, :] = 1.0 if r0 + R < H else 0.0     # conv2 row R

    # rgb halo rows r0-4 .. r0+R+4, col j = img x + 8
    rgbp = np.zeros((3, R + 8, WPAD), np.float32)
    lo2, hi2 = r0 - 4, r0 + R + 4
    slo2, shi2 = max(lo2, 0), min(hi2, H)
    rgbp[:, slo2 - lo2:shi2 - lo2, 8:8 + W] = rgb_b[:, slo2:shi2, :]
    sr = min(SR, R)
    nstrip = R // sr
    outs = []
    for shift in (0, 1):
        arr = np.zeros((nstrip * 128, RGBF), np.float32)
        for s_ in range(nstrip):
            for dy in range(9):
                rows = rgbp[:, s_ * sr + dy:s_ * sr + dy + sr, :]
                for h in range(2):
                    seg = rows[:, :, h * HW2 + 4 + shift:
                               h * HW2 + 4 + shift + 200]
                    arr_view = arr[s_ * 128 + h:s_ * 128 + 2 * sr + h:2]
                    for c in range(3):
                        arr_view[:, c * 1800 + dy * 200:
                                 c * 1800 + (dy + 1) * 200] = seg[c]
        outs.append(arr.astype(ml_dtypes.bfloat16))
    return x74d, outs[0], outs[1], maskd


def _prep_inputs(rgb, depth, w1, b1, w2, b2, w3, b3):
    R = H // 2
    x = np.concatenate([rgb, depth], axis=1)  # (B,4,H,W)
    wtsb, wtsr, wtb = prep_weights(w1, b1, w2, b2, w3, b3)
    in_maps = []
    for core in range(NC_):
        bi, half = divmod(core, 2)
        r0 = half * R
        x74d, rgbsA, rgbsB, maskd = prep_shard(x[bi], rgb[bi], r0, R)
        wtbc = wtb.copy()
        if r0 - 1 < 0:
            wtbc[0:64, 2] = 0.0       # conv2 row -1 out of image
        if r0 + R >= H:
            wtbc[64:128, 3] = 0.0     # conv2 row R out of image
        in_maps.append({"x74d": x74d, "rgbsA": rgbsA, "rgbsB": rgbsB,
                        "wtsb": wtsb, "wtsr": wtsr, "wtb": wtbc,
                        "maskd": maskd})
    return in_maps


_CACHE = {}


def _get_program(R=H // 2):
    if R not in _CACHE:
        _CACHE[R] = build_core_program(R)
    return _CACHE[R]


def kernel(rgb, depth, w1, b1, w2, b2, w3, b3):
    from concourse.bass_utils import run_bass_kernel_spmd
    rgb = np.asarray(rgb, np.float32)
    depth = np.asarray(depth, np.float32)
    nc = _get_program()
    in_maps = _prep_inputs(rgb, depth, np.asarray(w1, np.float32),
                           np.asarray(b1, np.float32),
                           np.asarray(w2, np.float32),
                           np.asarray(b2, np.float32),
                           np.asarray(w3, np.float32),
                           np.asarray(b3, np.float32))
    res = run_bass_kernel_spmd(nc, in_maps, core_ids=list(range(NC_)),
                               trace=bool(int(os.environ.get("KT_TRACE",
                                                             "0"))))
    R = H // 2
    outp = np.zeros((B, 3, H, W), np.float32)
    for core in range(NC_):
        bi, half = divmod(core, 2)
        outp[bi, :, half * R:(half + 1) * R, :] = res.results[core]["out"]
    kernel.last_result = res
    return outp


if __name__ == "__main__":
    nc = build_core_program(R=int(sys.argv[1]) if len(sys.argv) > 1 else 8)
    print("built ok")


# revision 50
# speedup vs baseline: 1.0238x; 1.0238x over previous
"""DepthAwareBokehDFN Trainium2 kernel (v2: row-pair matmuls).

Network (per image): x = concat(rgb, depth) (4ch) -> conv3x3(64)+relu ->
conv3x3(64)+relu -> conv3x3(81) -> softmax over 81 taps -> 9x9 dynamic
filtering of rgb.

Distribution: pure data parallel over 8 cores; shard = (batch, H-half),
192 output rows per core.  Halos recomputed from DRAM (no collectives).

v2 dataflow (row-PAIR streamed; all conv outputs produced two rows per
PSUM tile, M = 64ch x 2rows = 128):
  - conv1: host-side im2col with the pair structure baked in (x74: 36
    taps x 2 row-offsets + 2 bias/ones channels) -> ONE matmul per row
    pair (K=74, N=384).  Out-of-image rows are zeroed host-side
    (including the ones channel), so padding rows come out exactly 0.
  - conv2: 6 matmuls per row pair (3 kw x 2 input row-pairs, K=128);
    output partitions (co, j) land both window halves of one slot in a
    single ACT evac -> the per-row window-fill DMAs of v1 are gone.
    Image-boundary output rows (-1 / R) are zeroed by a per-core mask
    multiply (DVE), restoring SAME-pad semantics.
  - conv3: per row, 3 full-K (paired rows) + 3 half-K matmuls into a
    2-row PSUM tile; one bulk 2-row exp evac (ACT, bias=b3).
  - softmax denominator folded into the bokeh tree as a 4th channel.
  - bokeh in "y-layout" (partition = half-row), rgb prescattered on the
    host with per-tap shifts so every tap is a free-dim offset; products
    on DVE in bf16 (2x mode), pairwise adder tree (L1/L2 bf16, rest
    fp32); E reshaped into y-layout by per-row scatter DMAs (SWDGE).
"""

import os
import sys
import numpy as np

if "/opt/trn_rl_repo" not in sys.path:
    sys.path.insert(0, "/opt/trn_rl_repo")

import ml_dtypes  # noqa: E402
import concourse.bass as bass  # noqa: E402
import concourse.bacc as bacc  # noqa: E402
import concourse.mybir as mybir  # noqa: E402
import concourse.tile as tile  # noqa: E402

F32 = mybir.dt.float32
F32R = mybir.dt.float32r
BF16 = mybir.dt.bfloat16

B, H, W = 4, 384, 384
NC_ = 8         # cores
RS = 392        # row slot stride (elements) in window / x74 buffers
WPAD = 400      # rgb halo padded width
HW2 = 192       # half-row width
SR = 64         # rows per bokeh strip (=> 128 half-rows = 128 partitions)
NS = 6          # pair slots in h1w/h2w rings
XS = 8          # pair slots in x74 window
RGBF = 27 * 200  # rgb halo block elems per partition (3ch * 9dy * 200)

# bf16 weight table columns: l1 (74,128) | l2lo[3] | l2hi[3] (each 128x128)
C_L1 = 0
C_L2LO = 128
C_L2HI = C_L2LO + 384
WB_COLS = C_L2HI + 384          # 896
# f32r table: l3 full-K [even|odd][kw] 6x(128,81) then half-K:
#   even rows: kh2 at parts 0:64 (3x81); odd rows: kh0 at parts 64:128
C_L3F = 0
C_L3HE = 486
C_L3HO = 486 + 243
WR_COLS = 486 + 486             # 972


def build_core_program(R=192):
    """Builds the per-core Bass program.  R = output rows per core."""
    assert R % 2 == 0
    sr = min(SR, R)
    assert R % sr == 0
    nstrip = R // sr
    NP = (R + 4) // 2            # conv1 pairs: y = 2p-2, p in 0..NP-1

    nc = bacc.Bacc("TRN2", debug=False, enable_asserts=False,
                   num_devices=NC_, enable_partition_id=False,
                   num_swdge_queues=4)

    x74d = nc.dram_tensor("x74d", [74, NP, RS], BF16,
                          kind="ExternalInput").ap()
    rgbsA = nc.dram_tensor("rgbsA", [nstrip * 128, RGBF], BF16,
                           kind="ExternalInput").ap()
    rgbsB = nc.dram_tensor("rgbsB", [nstrip * 128, RGBF], BF16,
                           kind="ExternalInput").ap()
    wtsb = nc.dram_tensor("wtsb", [128, WB_COLS], BF16,
                          kind="ExternalInput").ap()
    wtsr = nc.dram_tensor("wtsr", [128, WR_COLS], BF16,
                          kind="ExternalInput").ap()
    wtb = nc.dram_tensor("wtb", [128, 4], F32, kind="ExternalInput").ap()
    maskd = nc.dram_tensor("maskd", [128, 384], F32,
                           kind="ExternalInput").ap()
    out = nc.dram_tensor("out", [3, R, W], F32, kind="ExternalOutput").ap()

    def sig1(y):   # h1w pair-slot of conv1 pair starting at even row y
        return ((y + 2) // 2) % NS

    def sig2(w0):  # h2w pair-slot of conv2 pair starting at odd row w0
        return ((w0 + 1) // 2) % NS

    AF = mybir.ActivationFunctionType

    with tile.TileContext(nc) as tc:
        with (
            tc.tile_pool(name="singles", bufs=1) as singles,
            tc.tile_pool(name="estg_pool", bufs=12) as estg_pool,
            tc.tile_pool(name="outstg_pool", bufs=1) as outstg_pool,
            tc.tile_pool(name="psum", bufs=1, space="PSUM") as psum,
        ):
            # ---- persistent SBUF state ----
            wtsb_sb = singles.tile([128, WB_COLS], BF16)
            nc.sync.dma_start(out=wtsb_sb, in_=wtsb)
            wtsr_sb = singles.tile([128, WR_COLS], BF16)
            nc.sync.dma_start(out=wtsr_sb, in_=wtsr)
            wtb_sb = singles.tile([128, 4], F32)
            nc.sync.dma_start(out=wtb_sb, in_=wtb)
            mask_sb = singles.tile([128, 384], F32)
            nc.sync.dma_start(out=mask_sb, in_=maskd)
            h1w = singles.tile([128, NS * RS], BF16)
            h2w = singles.tile([128, NS * RS], BF16)
            x74w = singles.tile([74, XS * RS], BF16)
            ebuf = [singles.tile([128, 81 * HW2], BF16, name=f"ebuf{i}")
                    for i in range(2)]
            rgbAB = [(singles.tile([128, RGBF], BF16, name=f"rgbA{i}"),
                      singles.tile([128, RGBF], BF16, name=f"rgbB{i}"))
                     for i in range(2)]
            tmpP = singles.tile([128, 81 * HW2], BF16)
            scrA = singles.tile([128, 40 * HW2], BF16)
            scrF = singles.tile([128, 19, HW2], F32)
            uacc = singles.tile([128, 4, HW2], F32)

            nc.vector.memset(h1w, 0.0)
            nc.vector.memset(h2w, 0.0)

            # weight slices
            l1 = wtsb_sb[0:74, C_L1:C_L1 + 128]
            l2lo = [wtsb_sb[0:128, C_L2LO + 128 * k:C_L2LO + 128 * (k + 1)]
                    for k in range(3)]
            l2hi = [wtsb_sb[0:128, C_L2HI + 128 * k:C_L2HI + 128 * (k + 1)]
                    for k in range(3)]
            # conv3 full-K: po=0 (even v), po=1 (odd v)
            l3f = [[wtsr_sb[0:128, C_L3F + 81 * (3 * po + k):
                            C_L3F + 81 * (3 * po + k + 1)]
                    for k in range(3)] for po in range(2)]
            l3he = [wtsr_sb[0:64, C_L3HE + 81 * k:C_L3HE + 81 * (k + 1)]
                    for k in range(3)]
            l3ho = [wtsr_sb[64:128, C_L3HO + 81 * k:C_L3HO + 81 * (k + 1)]
                    for k in range(3)]

            b2 = wtb_sb[0:128, 0:1]
            b3 = wtb_sb[0:81, 1:2]
            b2top = wtb_sb[0:128, 2:3]   # b2 with row -1 (parts 0:64) masked
            b2bot = wtb_sb[0:128, 3:4]   # b2 with row R (parts 64:128) masked

            # ---------------- emission helpers ----------------
            def emit_x74_batch(p0):
                # load x74 pair-slots p0..p0+3 in one DMA
                n = min(4, NP - p0)
                F = XS * RS
                dst = bass.AP(tensor=x74w.tensor, offset=(p0 % XS) * RS,
                              ap=[[F, 74], [RS, n], [1, RS]])
                src = bass.AP(tensor=x74d.tensor, offset=p0 * RS,
                              ap=[[NP * RS, 74], [RS, n], [1, RS]])
                # scalar queue: x74 loads never carry long waits, and
                # the sync queue may be parked on an out-DMA's RAW wait
                nc.scalar.dma_start(out=dst, in_=src)

            def emit_reshape(v, estg, idx):
                # scatter E row v (81, 384) bf16 into ebuf strip layout;
                # one DMA per half-row (dst = one partition)
                s, p0 = v // sr, 2 * (v % sr)
                eb = ebuf[s % 2]
                for h, eng in ((0, nc.scalar), (1, nc.gpsimd)):
                    eng.dma_start(
                        out=eb[p0 + h:p0 + h + 1, :],
                        in_=estg[0:81, idx, h * HW2:(h + 1) * HW2])

            def emit_rgb_dma(s):
                # double-buffered: HWDGE DMAs must never carry a long
                # wait (a parked DMA poisons its shared DMAHW sem lane
                # for every other waiter on that lane)
                ta, tb = rgbAB[s % 2]
                for t, srct in ((ta, rgbsA), (tb, rgbsB)):
                    nc.sync.dma_start(
                        out=t[0:2 * sr, :],
                        in_=srct[s * 128:s * 128 + 2 * sr, :])

            def emit_bokeh(s):
                np_ = 2 * sr  # partitions used
                rgbA, rgbB = rgbAB[s % 2]
                eb = ebuf[s % 2]
                EB = 81 * HW2
                SA = 40 * HW2
                ostg = outstg_pool.tile([128, 3, HW2], F32, name=f"ostg{s}",
                                        tag="ostg")

                with nc.allow_low_precision("bokeh bf16 tree by design"):
                    for ch in range(4):
                        if ch < 3:
                            # products tmpP[(dy,dx),x] = E * rgb_shift
                            for par, t in ((0, rgbA), (1, rgbB)):
                                tn = 5 - par  # 5 even-dx taps, 4 odd
                                dst = bass.AP(
                                    tensor=tmpP.tensor, offset=par * HW2,
                                    ap=[[EB, np_], [9 * HW2, 9],
                                        [2 * HW2, tn], [1, HW2]])
                                ein = bass.AP(
                                    tensor=eb.tensor, offset=par * HW2,
                                    ap=[[EB, np_], [9 * HW2, 9],
                                        [2 * HW2, tn], [1, HW2]])
                                rin = bass.AP(
                                    tensor=t.tensor, offset=ch * 1800,
                                    ap=[[RGBF, np_], [200, 9], [2, tn],
                                        [1, HW2]])
                                nc.vector.tensor_mul(dst, ein, rin)
                            src_t, SRCF = tmpP, EB
                        else:
                            src_t, SRCF = eb, EB

                        # pairwise tree over the 81 tap planes:
                        # L1: (0..79) -> scrA 0..39          (bf16)
                        nc.vector.tensor_add(
                            bass.AP(tensor=scrA.tensor, offset=0,
                                    ap=[[SA, np_], [HW2, 40], [1, HW2]]),
                            bass.AP(tensor=src_t.tensor, offset=0,
                                    ap=[[SRCF, np_], [2 * HW2, 40],
                                        [1, HW2]]),
                            bass.AP(tensor=src_t.tensor, offset=HW2,
                                    ap=[[SRCF, np_], [2 * HW2, 40],
                                        [1, HW2]]))
                        # L2: scrA 0..39 -> tmpP 0..19       (bf16)
                        nc.vector.tensor_add(
                            bass.AP(tensor=tmpP.tensor, offset=0,
                                    ap=[[EB, np_], [HW2, 20], [1, HW2]]),
                            bass.AP(tensor=scrA.tensor, offset=0,
                                    ap=[[SA, np_], [2 * HW2, 20], [1, HW2]]),
                            bass.AP(tensor=scrA.tensor, offset=HW2,
                                    ap=[[SA, np_], [2 * HW2, 20], [1, HW2]]))
                        # L3: tmpP 0..19 -> scrF 0..9        (fp32 out)
                        SF = 19 * HW2
                        nc.vector.tensor_add(
                            bass.AP(tensor=scrF.tensor, offset=0,
                                    ap=[[SF, np_], [HW2, 10], [1, HW2]]),
                            bass.AP(tensor=tmpP.tensor, offset=0,
                                    ap=[[EB, np_], [2 * HW2, 10], [1, HW2]]),
                            bass.AP(tensor=tmpP.tensor, offset=HW2,
                                    ap=[[EB, np_], [2 * HW2, 10], [1, HW2]]))
                        # L4: scrF 0..9 -> scrF 10..14
                        nc.vector.tensor_add(
                            bass.AP(tensor=scrF.tensor, offset=10 * HW2,
                                    ap=[[SF, np_], [HW2, 5], [1, HW2]]),
                            bass.AP(tensor=scrF.tensor, offset=0,
                                    ap=[[SF, np_], [2 * HW2, 5], [1, HW2]]),
                            bass.AP(tensor=scrF.tensor, offset=HW2,
                                    ap=[[SF, np_], [2 * HW2, 5], [1, HW2]]))
                        # L5: scrF 10..13 -> scrF 15..16  (leftover 14)
                        nc.vector.tensor_add(
                            bass.AP(tensor=scrF.tensor, offset=15 * HW2,
                                    ap=[[SF, np_], [HW2, 2], [1, HW2]]),
                            bass.AP(tensor=scrF.tensor, offset=10 * HW2,
                                    ap=[[SF, np_], [2 * HW2, 2], [1, HW2]]),
                            bass.AP(tensor=scrF.tensor, offset=11 * HW2,
                                    ap=[[SF, np_], [2 * HW2, 2], [1, HW2]]))
                        # L6: 15+16 -> 17 ; L7: 17+14 -> 18
                        nc.vector.tensor_add(scrF[0:np_, 17, :],
                                             scrF[0:np_, 15, :],
                                             scrF[0:np_, 16, :])
                        nc.vector.tensor_add(scrF[0:np_, 18, :],
                                             scrF[0:np_, 17, :],
                                             scrF[0:np_, 14, :])
                        # L8: + plane 80 (bf16 leftover) -> uacc[ch]
                        last = bass.AP(tensor=src_t.tensor, offset=80 * HW2,
                                       ap=[[SRCF, np_], [1, HW2]])
                        nc.vector.tensor_add(uacc[0:np_, ch, :],
                                             scrF[0:np_, 18, :], last)

                    # out = U * (1/S)
                    nc.vector.reciprocal(uacc[0:np_, 3, :], uacc[0:np_, 3, :])
                    for ch in range(3):
                        nc.vector.tensor_mul(ostg[0:np_, ch, :],
                                             uacc[0:np_, ch, :],
                                             uacc[0:np_, 3, :])

                # DMA strip output to DRAM (one DMA per channel).
                # SWDGE (gpsimd): its bokeh-long RAW wait must not park
                # an HWDGE ring/sem lane shared with the scatters; the
                # brief gpsimd-ring blockage is absorbed by the deep
                # estg pool.
                for ch in range(3):
                    dst = bass.AP(tensor=out.tensor,
                                  offset=ch * R * W + s * sr * W,
                                  ap=[[W, sr], [HW2, 2], [1, HW2]])
                    srcap = bass.AP(tensor=ostg.tensor, offset=ch * HW2,
                                    ap=[[3 * HW2, np_], [1, HW2]])
                    nc.gpsimd.dma_start(out=dst, in_=srcap)

            # ---------------- main row-pair loop ----------------
            # deep skew: conv2 lags conv1 by 2 pair-iterations, conv3
            # lags conv2 by 2 more, so each stage's inputs were evac'd
            # >=2 iterations ago and PE never waits on the previous
            # pair's ACT evac
            for k in range((R + 16) // 2):
                y = 2 * k - 2           # conv1 pair (y, y+1)
                if k == 16:
                    # strip-0 rgb prefetch: emitted well after the
                    # pipeline-fill window (its 2.76MB congests the
                    # SDMA engines and delays the completion receipts
                    # of the startup-critical DMAs by ~7us), yet ~90us
                    # before bokeh(0) needs the data at pair ~38
                    for t_, srct in ((rgbAB[0][0], rgbsA),
                                     (rgbAB[0][1], rgbsB)):
                        nc.gpsimd.dma_start(out=t_[0:2 * sr, :],
                                            in_=srct[0:2 * sr, :])
                if y <= R:
                    if k == 0:
                        emit_x74_batch(0)
                    if k % 4 == 0 and k + 4 < NP:
                        emit_x74_batch(k + 4)
                    ps1 = psum.tile([128, 512], F32, tag=f"c1{k % 2}",
                                    name=f"c1_{k}")
                    rhs = x74w[0:74, (k % XS) * RS + 1:(k % XS) * RS + 385]
                    nc.tensor.matmul(out=ps1[0:128, 0:384], lhsT=l1,
                                     rhs=rhs, start=True, stop=True)
                    nc.scalar.activation(
                        out=h1w[0:128, sig1(y) * RS + 1:sig1(y) * RS + 385],
                        in_=ps1[0:128, 0:384], func=AF.Relu)

                # conv2 pair (w0, w0+1), w0 odd
                w0 = y - 5
                if -1 <= w0 <= R - 1:
                    ps2 = psum.tile([128, 512], F32, tag=f"c2{k % 2}",
                                    name=f"c2_{k}")
                    slo, shi = sig1(w0 - 1), sig1(w0 + 1)
                    for kw in range(3):
                        nc.tensor.matmul(
                            out=ps2[0:128, 0:384], lhsT=l2lo[kw],
                            rhs=h1w[0:128, slo * RS + kw:slo * RS + kw + 384],
                            start=(kw == 0), stop=False)
                    for kw in range(3):
                        nc.tensor.matmul(
                            out=ps2[0:128, 0:384], lhsT=l2hi[kw],
                            rhs=h1w[0:128, shi * RS + kw:shi * RS + kw + 384],
                            start=False, stop=(kw == 2))
                    # image-boundary SAME-pad fix: mask rows -1 / R in
                    # PSUM before the evac (bias also masked, so padding
                    # rows come out exactly relu(0) = 0)
                    bias2 = b2
                    if w0 == -1:
                        nc.vector.tensor_mul(ps2[0:64, 0:384],
                                             ps2[0:64, 0:384],
                                             mask_sb[0:64, 0:384])
                        bias2 = b2top
                    if w0 == R - 1:
                        nc.vector.tensor_mul(ps2[64:128, 0:384],
                                             ps2[64:128, 0:384],
                                             mask_sb[64:128, 0:384])
                        bias2 = b2bot
                    s2 = sig2(w0)
                    nc.scalar.activation(
                        out=h2w[0:128, s2 * RS + 1:s2 * RS + 385],
                        in_=ps2[0:128, 0:384], func=AF.Relu, bias=bias2)

                # conv3 rows (y-12, y-11)
                v0 = y - 12
                if 0 <= v0 <= R - 2:
                    ps3 = psum.tile([81, 2, 512], F32, tag=f"c3{k % 2}",
                                    name=f"c3_{k}")
                    for idx, v in enumerate((v0, v0 + 1)):
                        if v % sr == 0 and v > 0:
                            emit_rgb_dma(v // sr)
                        po = v % 2
                        pstart = v - 1 + po   # full-K pair start (odd)
                        sf = sig2(pstart)
                        outap = ps3[0:81, idx, 0:384]
                        for kw in range(3):
                            nc.tensor.matmul(
                                out=outap, lhsT=l3f[po][kw],
                                rhs=h2w[0:128,
                                        sf * RS + kw:sf * RS + kw + 384],
                                start=(kw == 0), stop=False)
                        if po == 0:   # even v: kh2 = row v+1 at lo half
                            sh = sig2(v + 1)
                            for kw in range(3):
                                nc.tensor.matmul(
                                    out=outap, lhsT=l3he[kw],
                                    rhs=h2w[0:64, sh * RS + kw:
                                            sh * RS + kw + 384],
                                    start=False, stop=(kw == 2))
                        else:         # odd v: kh0 = row v-1 at hi half
                            sh = sig2(v - 2)
                            for kw in range(3):
                                nc.tensor.matmul(
                                    out=outap, lhsT=l3ho[kw],
                                    rhs=h2w[64:128, sh * RS + kw:
                                            sh * RS + kw + 384],
                                    start=False, stop=(kw == 2))
                    estg = estg_pool.tile([81, 2, 384], BF16,
                                          name=f"estg_{k}", tag="estg")
                    nc.scalar.activation(out=estg[0:81, :, :],
                                         in_=ps3[0:81, 0:2, 0:384],
                                         func=AF.Exp, bias=b3)
                    for idx, v in enumerate((v0, v0 + 1)):
                        emit_reshape(v, estg, idx)
                        if v % sr == sr - 1:
                            emit_bokeh(v // sr)

    nc.compile()
    return nc


# ------------------------- host side -------------------------

def prep_weights(w1, b1, w2, b2, w3, b3):
    wtsb = np.zeros((128, WB_COLS), np.float32)
    # l1: K = (t, j) 72 + 2 bias channels; M = (co, j')
    for kh in range(3):
        for kw in range(3):
            for c in range(4):
                t = kh * 12 + kw * 4 + c
                wtsb[t, C_L1:C_L1 + 64] = w1[:, c, kh, kw]
                wtsb[36 + t, C_L1 + 64:C_L1 + 128] = w1[:, c, kh, kw]
    wtsb[72, C_L1:C_L1 + 64] = b1
    wtsb[73, C_L1 + 64:C_L1 + 128] = b1
    # conv2: lo pair (w0-1, w0): kh = a - j; hi pair (w0+1, w0+2): a - j + 2
    for kw in range(3):
        lo = np.zeros((128, 128), np.float32)
        hi = np.zeros((128, 128), np.float32)
        for a in range(2):
            for j in range(2):
                khl = a - j
                if 0 <= khl <= 2:
                    lo[a * 64:(a + 1) * 64, j * 64:(j + 1) * 64] = \
                        w2[:, :, khl, kw].T
                khh = a - j + 2
                if 0 <= khh <= 2:
                    hi[a * 64:(a + 1) * 64, j * 64:(j + 1) * 64] = \
                        w2[:, :, khh, kw].T
        wtsb[:, C_L2LO + 128 * kw:C_L2LO + 128 * (kw + 1)] = lo
        wtsb[:, C_L2HI + 128 * kw:C_L2HI + 128 * (kw + 1)] = hi
    # conv3 f32r tables
    wtsr = np.zeros((128, WR_COLS), np.float32)
    for kw in range(3):
        # even v: pair (v-1, v): kh = a
        c0 = C_L3F + 81 * kw
        wtsr[0:64, c0:c0 + 81] = w3[:, :, 0, kw].T
        wtsr[64:128, c0:c0 + 81] = w3[:, :, 1, kw].T
        # odd v: pair (v, v+1): kh = a + 1
        c1 = C_L3F + 81 * (3 + kw)
        wtsr[0:64, c1:c1 + 81] = w3[:, :, 1, kw].T
        wtsr[64:128, c1:c1 + 81] = w3[:, :, 2, kw].T
        # half-K: even v: kh2 at parts 0:64; odd v: kh0 at parts 64:128
        wtsr[0:64, C_L3HE + 81 * kw:C_L3HE + 81 * (kw + 1)] = \
            w3[:, :, 2, kw].T
        wtsr[64:128, C_L3HO + 81 * kw:C_L3HO + 81 * (kw + 1)] = \
            w3[:, :, 0, kw].T
    wtb = np.zeros((128, 4), np.float32)
    wtb[0:64, 0] = b2
    wtb[64:128, 0] = b2
    wtb[0:81, 1] = b3
    wtb[:, 2] = wtb[:, 0]
    wtb[:, 3] = wtb[:, 0]
    return (wtsb.astype(ml_dtypes.bfloat16),
            wtsr.astype(ml_dtypes.bfloat16), wtb)


def prep_shard(x, rgb_b, r0, R):
    """x: (4,H,W) fp32 of one image; rgb_b: (3,H,W).

    Returns (x74d, rgbsA, rgbsB, maskd)."""
    NP = (R + 4) // 2
    # padded x rows r0-3 .. r0+R+3, width 392 (img col x at 1+x)
    xp = np.zeros((4, R + 6, RS), np.float32)
    lo, hi = r0 - 3, r0 + R + 3
    slo, shi = max(lo, 0), min(hi, H)
    xp[:, slo - lo:shi - lo, 1:385] = x[:, slo:shi, :]
    # x36 rows j = conv1 out rows -2 .. R+1 (index j+2)
    x36 = np.zeros((36, R + 4, RS), np.float32)
    for kh in range(3):
        for kw in range(3):
            blk = np.zeros((4, R + 4, RS), np.float32)
            if kw == 0:
                blk[:, :, 1:] = xp[:, kh:kh + R + 4, :-1]
            elif kw == 1:
                blk[:, :, :] = xp[:, kh:kh + R + 4, :]
            else:
                blk[:, :, :-1] = xp[:, kh:kh + R + 4, 1:]
            for c in range(4):
                x36[kh * 12 + kw * 4 + c] = blk[c]
    # in-image mask for conv1 out rows (global row in [0, H))
    def inimg(j):  # j = local conv1-out row
        g = r0 + j
        return 0 <= g < H
    x74 = np.zeros((74, NP, RS), np.float32)
    for p in range(NP):
        for j in range(2):
            row = 2 * p - 2 + j
            if inimg(row):
                x74[36 * j:36 * (j + 1), p, :] = x36[:, row + 2, :]
                x74[72 + j, p, 1:385] = 1.0
    x74d = x74.astype(ml_dtypes.bfloat16)

    maskd = np.zeros((128, 384), np.float32)
    maskd[0:64, :] = 1.0 if r0 - 1 >= 0 else 0.0      # conv2 row -1
    maskd[64:128, :] = 1.0 if r0 + R < H else 0.0     # conv2 row R

    # rgb halo rows r0-4 .. r0+R+4, col j = img x + 8
    rgbp = np.zeros((3, R + 8, WPAD), np.float32)
    lo2, hi2 = r0 - 4, r0 + R + 4
    slo2, shi2 = max(lo2, 0), min(hi2, H)
    rgbp[:, slo2 - lo2:shi2 - lo2, 8:8 + W] = rgb_b[:, slo2:shi2, :]
    sr = min(SR, R)
    nstrip = R // sr
    outs = []
    for shift in (0, 1):
        arr = np.zeros((nstrip * 128, RGBF), np.float32)
        for s_ in range(nstrip):
            for dy in range(9):
                rows = rgbp[:, s_ * sr + dy:s_ * sr + dy + sr, :]
                for h in range(2):
                    seg = rows[:, :, h * HW2 + 4 + shift:
                               h * HW2 + 4 + shift + 200]
                    arr_view = arr[s_ * 128 + h:s_ * 128 + 2 * sr + h:2]
                    for c in range(3):
                        arr_view[:, c * 1800 + dy * 200:
                                 c * 1800 + (dy + 1) * 200] = seg[c]
        outs.append(arr.astype(ml_dtypes.bfloat16))
    return x74d, outs[0], outs[1], maskd


def _prep_inputs(rgb, depth, w1, b1, w2, b2, w3, b3):
    R = H // 2
    x = np.concatenate([rgb, depth], axis=1)  # (B,4,H,W)
    wtsb, wtsr, wtb = prep_weights(w1, b1, w2, b2, w3, b3)
    in_maps = []
    for core in range(NC_):
        bi, half = divmod(core, 2)
        r0 = half * R
        x74d, rgbsA, rgbsB, maskd = prep_shard(x[bi], rgb[bi], r0, R)
        wtbc = wtb.copy()
        if r0 - 1 < 0:
            wtbc[0:64, 2] = 0.0       # conv2 row -1 out of image
        if r0 + R >= H:
            wtbc[64:128, 3] = 0.0     # conv2 row R out of image
        in_maps.append({"x74d": x74d, "rgbsA": rgbsA, "rgbsB": rgbsB,
                        "wtsb": wtsb, "wtsr": wtsr, "wtb": wtbc,
                        "maskd": maskd})
    return in_maps


_CACHE = {}


def _get_program(R=H // 2):
    if R not in _CACHE:
        _CACHE[R] = build_core_program(R)
    return _CACHE[R]


def kernel(rgb, depth, w1, b1, w2, b2, w3, b3):
    from concourse.bass_utils import run_bass_kernel_spmd
    rgb = np.asarray(rgb, np.float32)
    depth = np.asarray(depth, np.float32)
    nc = _get_program()
    in_maps = _prep_inputs(rgb, depth, np.asarray(w1, np.float32),
                           np.asarray(b1, np.float32),
                           np.asarray(w2, np.float32),
                           np.asarray(b2, np.float32),
                           np.asarray(w3, np.float32),
                           np.asarray(b3, np.float32))
    res = run_bass_kernel_spmd(nc, in_maps, core_ids=list(range(NC_)),
                               trace=bool(int(os.environ.get("KT_TRACE",
                                                             "0"))))
    R = H // 2
    outp = np.zeros((B, 3, H, W), np.float32)
    for core in range(NC_):
        bi, half = divmod(core, 2)
        outp[bi, :, half * R:(half + 1) * R, :] = res.results[core]["out"]
    kernel.last_result = res
    return outp


if __name__ == "__main__":
    nc = build_core_program(R=int(sys.argv[1]) if len(sys.argv) > 1 else 8)
    print("built ok")
